# revision 18
# baseline (speedup 1.0000x reference)
"""LoRA attention processor on 8 NeuronCores (Trainium2, Bass/Tile).

Reference computation (B=2, S=4096, D=1280, H=8 heads, dh=160, rank-4 LoRA
on K/V):
    q = x @ Wq; k = x @ Wk; v = x @ Wv
    k += (k @ Ak) @ Bk; v += (v @ Av) @ Bv        (LoRA, rank 4)
    attn = softmax(q k^T / sqrt(dh)) v   per head
    out = attn @ Wout + b_out

Sharding: core c handles batch b = c//4 and head pair p = c%4 (columns
320p:320p+320 of the QKV projections, rows of Wout). The LoRA update is
folded into the weights on the host: k + (k@Ak)@Bk == x @ (Wk + Wk@Ak@Bk).
Each core returns a partial output (its heads' contribution to attn@Wout);
the host sums the 4 partials per batch and adds the bias.

Projections/PV/out-proj run with bf16 inputs (fp32 PSUM accumulation).
The q k^T scores run as fp8e4 DoubleRow matmuls (2 k-subtiles per
instruction, 0.5 cycles/row): Q^T and K^T are stored as fp8 hi + lo
residual pairs in [128, 2(k-tile), S] layout (d 0:128 | d 128:160 zero-
padded to 128), and scores = qh*kh + ql*kh + qh*kl (the dropped lo*lo
term is ~1e-3).  Q is pre-scaled by 64 on the host so fp8 values avoid
the subnormal range; the exp compensates with its scale operand.

Attention runs transposed (scores^T[kpos, qpos]) so exp reads PSUM
directly and PV needs no transposes; the softmax denominator rides as a
ones-column of V (dv padded to 256 so every PV stationary is full
128x128 -- quadrant-mode matmuls break LDWEIGHTS prefetch).  Per
q-chunk the j-loop is software-pipelined (scores(j); PV(j-3); exp(j))
and the output projection of the previous q-chunk is emitted inside the
next j-loop, so the in-order PE never waits on the ACT/DVE chains.
"""

import numpy as np
import ml_dtypes
from contextlib import ExitStack

import concourse.bass as bass
import concourse.tile as tile
from concourse import bacc, mybir
from concourse.bass_utils import run_bass_kernel_spmd

B, S, D = 2, 4096, 1280
H, DH = 8, 160
HP = 320           # head-pair columns per core (2 heads)
N_CORES = 8
SC = 512           # free-dim chunk (q columns)
NSC = S // SC      # 8
CK = 128           # contraction chunk
NCK = D // CK      # 10
NJ = S // 128      # 32 k-position blocks
QSCALE = 64.0      # fp8 subnormal-avoidance scale on Q (undone in exp)
F32 = mybir.dt.float32
F32R = mybir.dt.float32r
BF16 = mybir.dt.bfloat16
FP8 = mybir.dt.float8e4

_CACHE = {}


def build():
    nc = bacc.Bacc("TRN2", target_bir_lowering=False, debug=False,
                   num_devices=N_CORES)
    # host pre-interleaves the contraction dim: [part, chunk, cols]
    xT = nc.dram_tensor("xT", [CK, NCK, S], BF16, kind="ExternalInput").ap()
    # [.., 0:128] = head0 d 0:128, [.., 128:256] = head1 d 0:128
    wqm = nc.dram_tensor("wqm", [CK, NCK, 256], BF16,
                         kind="ExternalInput").ap()
    wkm = nc.dram_tensor("wkm", [CK, NCK, 256], BF16,
                         kind="ExternalInput").ap()
    # packed 32-row tails: q_h0 | q_h1 | k_h0 | k_h1 (d 128:160 each head)
    wt = nc.dram_tensor("wt", [CK, NCK, 128], BF16, kind="ExternalInput").ap()
    wv = nc.dram_tensor("wv", [CK, NCK, HP], BF16, kind="ExternalInput").ap()
    wo = nc.dram_tensor("wo", [HP, D], BF16, kind="ExternalInput").ap()
    out = nc.dram_tensor("out", [S, D], BF16, kind="ExternalOutput").ap()

    Exp = mybir.ActivationFunctionType.Exp
    Copy = mybir.ActivationFunctionType.Copy
    DR = mybir.MatmulPerfMode.DoubleRow

    with tile.TileContext(nc) as tc, ExitStack() as top:
        kq_pool = top.enter_context(tc.tile_pool(name="kq", bufs=1))
        v_pool = top.enter_context(tc.tile_pool(name="vp", bufs=1))
        wop = top.enter_context(tc.tile_pool(name="wop", bufs=1))
        cst = top.enter_context(tc.tile_pool(name="cst", bufs=1))

        # fp8 hi/lo score operands: [d-part, k-tile, seq]; k-tile 0 = d
        # 0:128, k-tile 1 = d 128:160 + zero pad
        Q8H = [kq_pool.tile([128, 2, S], FP8, name=f"Q8H{h}", tag=f"Q8H{h}")
               for h in range(2)]
        Q8L = [kq_pool.tile([128, 2, S], FP8, name=f"Q8L{h}", tag=f"Q8L{h}")
               for h in range(2)]
        K8H = [kq_pool.tile([128, 2, S], FP8, name=f"K8H{h}", tag=f"K8H{h}")
               for h in range(2)]
        K8L = [kq_pool.tile([128, 2, S], FP8, name=f"K8L{h}", tag=f"K8L{h}")
               for h in range(2)]
        # V natural per head: [kpos-part, j, dv]; col 160 = ones (denom),
        # cols 161:256 = zero pad (keeps PV "B" stationary at M=128)
        V = [v_pool.tile([128, NJ, 256], BF16, name=f"V{h}", tag=f"V{h}")
             for h in range(2)]
        # output-projection chunks: rows = wo rows 0:128 / 160:288 /
        # (128:160 | 288:320)
        WOA = [wop.tile([128, D], BF16, name=f"WOA{h}", tag=f"WOA{h}")
               for h in range(2)]
        WOB = wop.tile([128, D], BF16, name="WOB", tag="WOB")
        nc.vector.memset(WOB[64:128, :], 0.0)
        ones2f = cst.tile([128, 128], F32, name="ones2f", tag="ones2f")
        nc.vector.memset(ones2f[:], 0.0)
        nc.vector.memset(ones2f[0:1, :], 1.0)
        ones2 = cst.tile([128, 128], F32R, name="ones2", tag="ones2")
        nc.vector.tensor_copy(ones2[:], ones2f[:])
        # denominator staging: row 0 is live, rows 1:128 stay zero so the
        # broadcast matmul keeps a full-mode K=128 stationary
        zf = cst.tile([128, SC], F32, name="zf", tag="zf")
        nc.vector.memset(zf[:], 0.0)
        den = [cst.tile([128, SC], F32R, name=f"den{h}", tag=f"den{h}")
               for h in range(2)]
        for h in range(2):
            nc.vector.tensor_copy(den[h][:], zf[:])
        # zero pads: fp8 k-tile-1 rows 32:128, V cols 161:256
        for h in range(2):
            for t in (Q8H[h], Q8L[h], K8H[h], K8L[h]):
                nc.vector.memset(t[32:64, 1, :], 0.0)
                nc.vector.memset(t[64:128, 1, :], 0.0)
            nc.vector.memset(V[h][:, :, 160:161], 1.0)
            nc.gpsimd.memset(V[h][:, :, 161:256], 0.0)

        # ---- phase 1: projections into SBUF-resident Q^T/K^T/V ----
        with ExitStack() as ph1:
            xp = ph1.enter_context(tc.tile_pool(name="xp", bufs=2))
            wp = ph1.enter_context(tc.tile_pool(name="wp", bufs=1))
            pqk = ph1.enter_context(tc.tile_pool(name="pqk", bufs=3,
                                                 space="PSUM"))
            pvp = ph1.enter_context(tc.tile_pool(name="pvp", bufs=2,
                                                 space="PSUM"))
            sp = ph1.enter_context(tc.tile_pool(name="sp", bufs=1))

            # warm the ACT exp table before phase 2 needs it
            warm = sp.tile([1, 2], F32, tag="warm")
            nc.vector.memset(warm[:], 0.0)
            warm2 = sp.tile([1, 2], F32, tag="warm2")
            nc.scalar.activation(warm2[:], warm[:], Exp)

            # first x chunk before the weights so matmuls start early
            xt0 = xp.tile([CK, NCK, SC], BF16, tag="xt", name="xt0")
            nc.sync.dma_start(xt0[:], xT[:, :, 0:SC])
            wts = {}
            for nm, src, w in (("wqm", wqm, 256), ("wkm", wkm, 256),
                               ("wt", wt, 128), ("wv", wv, HP)):
                t = wp.tile([CK, NCK, w], BF16, name=f"{nm}_t", tag=f"{nm}_t")
                nc.sync.dma_start(t[:], src[:])
                wts[nm] = t
            nc.sync.dma_start(WOA[0][:], wo[0:128, :])
            nc.sync.dma_start(WOA[1][:], wo[160:288, :])
            nc.sync.dma_start(WOB[0:32, :], wo[128:160, :])
            nc.sync.dma_start(WOB[32:64, :], wo[288:320, :])

            # main groups: (weight, col off, hi tile, lo tile)
            mains = [("wqm", 0, Q8H[0], Q8L[0]),
                     ("wqm", 128, Q8H[1], Q8L[1]),
                     ("wkm", 0, K8H[0], K8L[0]),
                     ("wkm", 128, K8H[1], K8L[1])]
            # tails: psum row off -> (hi, lo) target head tiles
            tails = [(0, Q8H[0], Q8L[0]), (32, Q8H[1], Q8L[1]),
                     (64, K8H[0], K8L[0]), (96, K8H[1], K8L[1])]
            for sc in range(NSC):
                qs = slice(sc * SC, (sc + 1) * SC)
                if sc == 0:
                    xt = xt0
                else:
                    xt = xp.tile([CK, NCK, SC], BF16, tag="xt", name="xt")
                    nc.sync.dma_start(xt[:], xT[:, :, qs])
                for nm, coff, hi, lo in mains:
                    ps = pqk.tile([128, SC], F32, tag="pqk")
                    for c in range(NCK):
                        nc.tensor.matmul(
                            ps[:], wts[nm][:, c, coff:coff + 128],
                            xt[:, c, :], start=(c == 0), stop=(c == NCK - 1))
                    nc.vector.tensor_copy(hi[:, 0, qs], ps[:])
                    nc.vector.tensor_sub(lo[:, 0, qs], ps[:], hi[:, 0, qs])
                pst = pqk.tile([128, SC], F32, tag="pqk")
                for c in range(NCK):
                    nc.tensor.matmul(
                        pst[:], wts["wt"][:, c, :], xt[:, c, :],
                        start=(c == 0), stop=(c == NCK - 1))
                for roff, hi, lo in tails:
                    nc.vector.tensor_copy(hi[0:32, 1, qs],
                                          pst[roff:roff + 32, :])
                    nc.vector.tensor_sub(lo[0:32, 1, qs],
                                         pst[roff:roff + 32, :],
                                         hi[0:32, 1, qs])
                # V natural: psum[s-part, dv] = x[c, s].T @ wv[c, :]
                for st4 in range(4):
                    s0 = sc * 4 + st4
                    ps = pvp.tile([128, HP], F32, tag="pv")
                    for c in range(NCK):
                        nc.tensor.matmul(
                            ps[:], xt[:, c, st4 * 128:(st4 + 1) * 128],
                            wts["wv"][:, c, :], start=(c == 0),
                            stop=(c == NCK - 1))
                    for h in range(2):
                        nc.scalar.activation(V[h][:, s0, 0:160],
                                             ps[:, h * 160:(h + 1) * 160],
                                             Copy)

        # ---- phase 2+3: attention + fused output projection ----
        with ExitStack() as ph2:
            scp = ph2.enter_context(tc.tile_pool(name="scp", bufs=2,
                                                 space="PSUM"))
            oap = ph2.enter_context(tc.tile_pool(name="oap", bufs=1,
                                                 space="PSUM"))
            ep = ph2.enter_context(tc.tile_pool(name="ep", bufs=3))
            nsb = ph2.enter_context(tc.tile_pool(name="nsb", bufs=2))

            pending_ph3 = []

            def emit_ph3(OTA, OTB, qc):
                chunks = [(OTA[0], WOA[0]), (OTA[1], WOA[1]), (OTB, WOB)]
                for st4 in range(4):
                    ss = slice(st4 * 128, (st4 + 1) * 128)
                    row = qc * SC + st4 * 128
                    ost = nsb.tile([128, D], BF16, tag="ost", name="ost")
                    p01 = scp.tile([128, 2, SC], F32, tag="sc", name="p01")
                    for i, (ot, w) in enumerate(chunks):
                        nc.tensor.matmul(p01[:, 0, :], ot[:, ss], w[:, 0:512],
                                         start=(i == 0), stop=(i == 2))
                    for i, (ot, w) in enumerate(chunks):
                        nc.tensor.matmul(p01[:, 1, :], ot[:, ss],
                                         w[:, 512:1024],
                                         start=(i == 0), stop=(i == 2))
                    p2t = scp.tile([128, 2, SC], F32, tag="sc", name="p2t")
                    for i, (ot, w) in enumerate(chunks):
                        nc.tensor.matmul(p2t[:, 0, 0:256], ot[:, ss],
                                         w[:, 1024:1280],
                                         start=(i == 0), stop=(i == 2))
                    nc.scalar.activation(ost[:, 0:512], p01[:, 0, :], Copy)
                    nc.scalar.activation(ost[:, 512:1024], p01[:, 1, :], Copy)
                    nc.vector.tensor_copy(ost[:, 1024:1280], p2t[:, 0, 0:256])
                    nc.sync.dma_start(out[row:row + 128, :], ost[:])

            for qc in range(NSC):
                qs = slice(qc * SC, (qc + 1) * SC)
                oa = [oap.tile([128, SC], F32, tag=f"oa{h}", name=f"oa{h}")
                      for h in range(2)]
                ob = [oap.tile([128, SC], F32, tag=f"ob{h}", name=f"ob{h}")
                      for h in range(2)]
                exs = {}

                def emit_pv(j):
                    for h in range(2):
                        nc.tensor.matmul(oa[h][:], V[h][:, j, 0:128],
                                         exs[j][:, h, :],
                                         start=(j == 0), stop=(j == NJ - 1))
                        nc.tensor.matmul(ob[h][:], V[h][:, j, 128:256],
                                         exs[j][:, h, :],
                                         start=(j == 0), stop=(j == NJ - 1))

                for j in range(NJ):
                    js = slice(j * 128, (j + 1) * 128)
                    sc_ps = scp.tile([128, 2, SC], F32, tag="sc", name="sc")
                    for h in range(2):
                        nc.tensor.matmul(sc_ps[:, h, :], K8H[h][:, :, js],
                                         Q8H[h][:, :, qs], perf_mode=DR,
                                         start=True, stop=False)
                        nc.tensor.matmul(sc_ps[:, h, :], K8L[h][:, :, js],
                                         Q8H[h][:, :, qs], perf_mode=DR,
                                         start=False, stop=False)
                        nc.tensor.matmul(sc_ps[:, h, :], K8H[h][:, :, js],
                                         Q8L[h][:, :, qs], perf_mode=DR,
                                         start=False, stop=True)
                    if j == 4 and pending_ph3:
                        emit_ph3(*pending_ph3.pop())
                    if j >= 3:
                        emit_pv(j - 3)
                        del exs[j - 3]
                    ex = ep.tile([128, 2, SC], BF16, tag="ex", name="ex")
                    nc.scalar.activation(ex[:], sc_ps[:], Exp,
                                         scale=1.0 / QSCALE)
                    exs[j] = ex
                for j in range(NJ - 3, NJ):
                    emit_pv(j)

                # normalization: broadcast 1/denom and scale into bf16
                # oT chunk tiles (rows match the WOA/WOB chunking)
                for h in range(2):
                    nc.vector.tensor_copy(den[h][0:1, :], ob[h][32:33, :])
                rbdt = scp.tile([128, 2, SC], F32, tag="sc", name="rbdt")
                for h in range(2):
                    nc.tensor.matmul(rbdt[:, h, :], ones2[:], den[h][:],
                                     start=True, stop=True)
                rbs = nsb.tile([128, 2, SC], F32, tag="rbs", name="rbs")
                nc.vector.reciprocal_approx_fast(rbs[:], rbdt[:])
                OTA = [nsb.tile([128, SC], BF16, tag=f"ota{h}",
                                name=f"OTA{h}") for h in range(2)]
                OTB = nsb.tile([128, SC], BF16, tag="otb", name="OTB")
                nc.vector.memset(OTB[64:128, :], 0.0)
                nc.vector.tensor_mul(OTA[0][:], oa[0][:], rbs[:, 0, :])
                nc.vector.tensor_mul(OTA[1][:], oa[1][:], rbs[:, 1, :])
                nc.vector.tensor_mul(OTB[0:32, :], ob[0][0:32, :],
                                     rbs[0:32, 0, :])
                nc.vector.tensor_mul(OTB[32:64, :], ob[1][0:32, :],
                                     rbs[0:32, 1, :])

                # output projection is deferred into the next q-chunk's
                # j-loop so the PE stays busy while DVE finishes the muls
                pending_ph3.append((OTA, OTB, qc))
            while pending_ph3:
                emit_ph3(*pending_ph3.pop())

    nc.compile()
    return nc


def _interleave(w):
    """[D, n] -> [128, 10, n] with out[p, c, :] = w[c*128 + p, :]."""
    n = w.shape[1]
    return np.ascontiguousarray(
        w.reshape(NCK, CK, n).transpose(1, 0, 2))


def kernel(hidden_states, w_q, w_k, w_v, lora_k_a, lora_k_b,
           lora_v_a, lora_v_b, w_out, b_out):
    f64 = np.float64
    bf16 = ml_dtypes.bfloat16
    wk_eff = (w_k.astype(f64)
              + w_k.astype(f64) @ lora_k_a.astype(f64) @ lora_k_b.astype(f64))
    wv_eff = (w_v.astype(f64)
              + w_v.astype(f64) @ lora_v_a.astype(f64) @ lora_v_b.astype(f64))
    wq_s = w_q.astype(f64) * (QSCALE / np.sqrt(DH))

    xT = [np.ascontiguousarray(np.asarray(hidden_states)[b].T.astype(bf16)
                               .reshape(NCK, CK, S).transpose(1, 0, 2))
          for b in range(B)]

    in_maps = []
    for c in range(N_CORES):
        b, p = c // 4, c % 4
        cols = slice(p * HP, (p + 1) * HP)
        wq_c = wq_s[:, cols]
        wk_c = wk_eff[:, cols]
        in_maps.append({
            "xT": xT[b],
            "wqm": _interleave(np.concatenate(
                [wq_c[:, 0:128], wq_c[:, 160:288]], axis=1).astype(bf16)),
            "wkm": _interleave(np.concatenate(
                [wk_c[:, 0:128], wk_c[:, 160:288]], axis=1).astype(bf16)),
            "wt": _interleave(np.concatenate(
                [wq_c[:, 128:160], wq_c[:, 288:320],
                 wk_c[:, 128:160], wk_c[:, 288:320]], axis=1).astype(bf16)),
            "wv": _interleave(wv_eff[:, cols].astype(bf16)),
            "wo": np.ascontiguousarray(
                w_out.astype(f64)[cols, :].astype(bf16)),
        })

    global _last_in_maps
    _last_in_maps = in_maps
    if "nc" not in _CACHE:
        _CACHE["nc"] = build()
    res = run_bass_kernel_spmd(_CACHE["nc"], in_maps, list(range(N_CORES)))

    out = np.zeros((B, S, D), np.float32)
    for c in range(N_CORES):
        out[c // 4] += res.results[c]["out"].astype(np.float32)
    out += np.asarray(b_out, np.float32)
    return out


# revision 19
# speedup vs baseline: 1.1949x; 1.1949x over previous
"""LoRA attention processor on 8 NeuronCores (Trainium2, Bass/Tile).

Reference computation (B=2, S=4096, D=1280, H=8 heads, dh=160, rank-4 LoRA
on K/V):
    q = x @ Wq; k = x @ Wk; v = x @ Wv
    k += (k @ Ak) @ Bk; v += (v @ Av) @ Bv        (LoRA, rank 4)
    attn = softmax(q k^T / sqrt(dh)) v   per head
    out = attn @ Wout + b_out

Sharding: core c handles batch b = c//4 and head pair p = c%4 (columns
320p:320p+320 of the QKV projections, rows of Wout). The LoRA update is
folded into the weights on the host: k + (k@Ak)@Bk == x @ (Wk + Wk@Ak@Bk).
Each core returns a partial output (its heads' contribution to attn@Wout);
the host sums the 4 partials per batch and adds the bias.

All matmul inputs are bf16 (fp32 PSUM accumulation).  Q^T/K^T live in
SBUF as per-head 128-row "A" tiles plus zero-padded 128-row "B" tail
tiles (d 128:160 + zeros); the four 32-row tails (q/k x h0/h1) are
produced by ONE projection matmul against a host-packed weight block.
All stationaries are full 128x128 -- quadrant-mode (K<128) matmuls
break LDWEIGHTS prefetch and cost ~50% extra.

Attention runs transposed (scores^T[kpos, qpos]) so exp reads PSUM
directly and PV needs no transposes; the softmax denominator rides as a
ones-column of V (dv padded to 256 so every PV stationary is full
128x128 -- quadrant-mode matmuls break LDWEIGHTS prefetch).  Per
q-chunk the j-loop is software-pipelined (scores(j); PV(j-3); exp(j))
and the output projection of the previous q-chunk is emitted inside the
next j-loop, so the in-order PE never waits on the ACT/DVE chains.
"""

import numpy as np
import ml_dtypes
from contextlib import ExitStack

import concourse.bass as bass
import concourse.tile as tile
from concourse import bacc, mybir
from concourse.bass_utils import run_bass_kernel_spmd

B, S, D = 2, 4096, 1280
H, DH = 8, 160
HP = 320           # head-pair columns per core (2 heads)
N_CORES = 8
SC = 512           # free-dim chunk (q columns)
NSC = S // SC      # 8
CK = 128           # contraction chunk
NCK = D // CK      # 10
NJ = S // 128      # 32 k-position blocks
QSCALE = 1.0
F32 = mybir.dt.float32
F32R = mybir.dt.float32r
BF16 = mybir.dt.bfloat16
FP8 = mybir.dt.float8e4

_CACHE = {}


def build():
    nc = bacc.Bacc("TRN2", target_bir_lowering=False, debug=False,
                   num_devices=N_CORES)
    # host pre-interleaves the contraction dim: [part, chunk, cols]
    xT = nc.dram_tensor("xT", [CK, NCK, S], BF16, kind="ExternalInput").ap()
    # [.., 0:128] = head0 d 0:128, [.., 128:256] = head1 d 0:128
    wqm = nc.dram_tensor("wqm", [CK, NCK, 256], BF16,
                         kind="ExternalInput").ap()
    wkm = nc.dram_tensor("wkm", [CK, NCK, 256], BF16,
                         kind="ExternalInput").ap()
    # packed 32-row tails: q_h0 | q_h1 | k_h0 | k_h1 (d 128:160 each head)
    wt = nc.dram_tensor("wt", [CK, NCK, 128], BF16, kind="ExternalInput").ap()
    wv = nc.dram_tensor("wv", [CK, NCK, HP], BF16, kind="ExternalInput").ap()
    wo = nc.dram_tensor("wo", [HP, D], BF16, kind="ExternalInput").ap()
    out = nc.dram_tensor("out", [S, D], BF16, kind="ExternalOutput").ap()

    Exp = mybir.ActivationFunctionType.Exp
    Copy = mybir.ActivationFunctionType.Copy
    DR = mybir.MatmulPerfMode.DoubleRow

    with tile.TileContext(nc) as tc, ExitStack() as top:
        kq_pool = top.enter_context(tc.tile_pool(name="kq", bufs=1))
        v_pool = top.enter_context(tc.tile_pool(name="vp", bufs=1))
        wop = top.enter_context(tc.tile_pool(name="wop", bufs=1))
        cst = top.enter_context(tc.tile_pool(name="cst", bufs=1))

        KTA = [kq_pool.tile([128, S], BF16, name=f"KTA{h}", tag=f"KTA{h}")
               for h in range(2)]
        KTB = [kq_pool.tile([128, S], BF16, name=f"KTB{h}", tag=f"KTB{h}")
               for h in range(2)]
        QTA = [kq_pool.tile([128, S], BF16, name=f"QTA{h}", tag=f"QTA{h}")
               for h in range(2)]
        QTB = [kq_pool.tile([128, S], BF16, name=f"QTB{h}", tag=f"QTB{h}")
               for h in range(2)]
        # V natural per head: [kpos-part, j, dv]; col 160 = ones (denom),
        # cols 161:256 = zero pad (keeps PV "B" stationary at M=128)
        V = [v_pool.tile([128, NJ, 256], BF16, name=f"V{h}", tag=f"V{h}")
             for h in range(2)]
        # output-projection chunks: rows = wo rows 0:128 / 160:288 /
        # (128:160 | 288:320)
        WOA = [wop.tile([128, D], BF16, name=f"WOA{h}", tag=f"WOA{h}")
               for h in range(2)]
        WOB = wop.tile([128, D], BF16, name="WOB", tag="WOB")
        nc.vector.memset(WOB[64:128, :], 0.0)
        ones2f = cst.tile([128, 128], F32, name="ones2f", tag="ones2f")
        nc.vector.memset(ones2f[:], 0.0)
        nc.vector.memset(ones2f[0:1, :], 1.0)
        ones2 = cst.tile([128, 128], F32R, name="ones2", tag="ones2")
        nc.vector.tensor_copy(ones2[:], ones2f[:])
        # denominator staging: row 0 is live, rows 1:128 stay zero so the
        # broadcast matmul keeps a full-mode K=128 stationary
        zf = cst.tile([128, SC], F32, name="zf", tag="zf")
        nc.vector.memset(zf[:], 0.0)
        den = [cst.tile([128, SC], F32R, name=f"den{h}", tag=f"den{h}")
               for h in range(2)]
        for h in range(2):
            nc.vector.tensor_copy(den[h][:], zf[:])
        # zero pads: B-tail rows 32:128, V cols 161:256
        for h in range(2):
            for t in (QTB[h], KTB[h]):
                nc.vector.memset(t[32:64, :], 0.0)
                nc.vector.memset(t[64:128, :], 0.0)
            nc.vector.memset(V[h][:, :, 160:161], 1.0)
            nc.gpsimd.memset(V[h][:, :, 161:256], 0.0)

        # ---- phase 1: projections into SBUF-resident Q^T/K^T/V ----
        with ExitStack() as ph1:
            xp = ph1.enter_context(tc.tile_pool(name="xp", bufs=2))
            wp = ph1.enter_context(tc.tile_pool(name="wp", bufs=1))
            pqk = ph1.enter_context(tc.tile_pool(name="pqk", bufs=3,
                                                 space="PSUM"))
            pvp = ph1.enter_context(tc.tile_pool(name="pvp", bufs=2,
                                                 space="PSUM"))
            sp = ph1.enter_context(tc.tile_pool(name="sp", bufs=1))

            # warm the ACT exp table before phase 2 needs it
            warm = sp.tile([1, 2], F32, tag="warm")
            nc.vector.memset(warm[:], 0.0)
            warm2 = sp.tile([1, 2], F32, tag="warm2")
            nc.scalar.activation(warm2[:], warm[:], Exp)

            # first x chunk before the weights so matmuls start early
            xt0 = xp.tile([CK, NCK, SC], BF16, tag="xt", name="xt0")
            nc.sync.dma_start(xt0[:], xT[:, :, 0:SC])
            wts = {}
            for nm, src, w in (("wqm", wqm, 256), ("wkm", wkm, 256),
                               ("wt", wt, 128), ("wv", wv, HP)):
                t = wp.tile([CK, NCK, w], BF16, name=f"{nm}_t", tag=f"{nm}_t")
                nc.sync.dma_start(t[:], src[:])
                wts[nm] = t
            nc.sync.dma_start(WOA[0][:], wo[0:128, :])
            nc.sync.dma_start(WOA[1][:], wo[160:288, :])
            nc.sync.dma_start(WOB[0:32, :], wo[128:160, :])
            nc.sync.dma_start(WOB[32:64, :], wo[288:320, :])

            # main groups: (weight, col off, dest tile)
            mains = [("wqm", 0, QTA[0]), ("wqm", 128, QTA[1]),
                     ("wkm", 0, KTA[0]), ("wkm", 128, KTA[1])]
            # tails: psum row off -> dest tile
            tails = [(0, QTB[0]), (32, QTB[1]), (64, KTB[0]), (96, KTB[1])]
            for sc in range(NSC):
                qs = slice(sc * SC, (sc + 1) * SC)
                if sc == 0:
                    xt = xt0
                else:
                    xt = xp.tile([CK, NCK, SC], BF16, tag="xt", name="xt")
                    nc.sync.dma_start(xt[:], xT[:, :, qs])
                for nm, coff, dst in mains:
                    ps = pqk.tile([128, SC], F32, tag="pqk")
                    for c in range(NCK):
                        nc.tensor.matmul(
                            ps[:], wts[nm][:, c, coff:coff + 128],
                            xt[:, c, :], start=(c == 0), stop=(c == NCK - 1))
                    nc.vector.tensor_copy(dst[:, qs], ps[:])
                pst = pqk.tile([128, SC], F32, tag="pqk")
                for c in range(NCK):
                    nc.tensor.matmul(
                        pst[:], wts["wt"][:, c, :], xt[:, c, :],
                        start=(c == 0), stop=(c == NCK - 1))
                for roff, dst in tails:
                    nc.vector.tensor_copy(dst[0:32, qs],
                                          pst[roff:roff + 32, :])
                # V natural: psum[s-part, dv] = x[c, s].T @ wv[c, :]
                for st4 in range(4):
                    s0 = sc * 4 + st4
                    ps = pvp.tile([128, HP], F32, tag="pv")
                    for c in range(NCK):
                        nc.tensor.matmul(
                            ps[:], xt[:, c, st4 * 128:(st4 + 1) * 128],
                            wts["wv"][:, c, :], start=(c == 0),
                            stop=(c == NCK - 1))
                    for h in range(2):
                        nc.scalar.activation(V[h][:, s0, 0:160],
                                             ps[:, h * 160:(h + 1) * 160],
                                             Copy)

        # ---- phase 2+3: attention + fused output projection ----
        with ExitStack() as ph2:
            scp = ph2.enter_context(tc.tile_pool(name="scp", bufs=2,
                                                 space="PSUM"))
            oap = ph2.enter_context(tc.tile_pool(name="oap", bufs=1,
                                                 space="PSUM"))
            ep = ph2.enter_context(tc.tile_pool(name="ep", bufs=3))
            nsb = ph2.enter_context(tc.tile_pool(name="nsb", bufs=2))

            pending_ph3 = []

            def emit_ph3(OTA, OTB, qc):
                chunks = [(OTA[0], WOA[0]), (OTA[1], WOA[1]), (OTB, WOB)]
                for st4 in range(4):
                    ss = slice(st4 * 128, (st4 + 1) * 128)
                    row = qc * SC + st4 * 128
                    ost = nsb.tile([128, D], BF16, tag="ost", name="ost")
                    p01 = scp.tile([128, 2, SC], F32, tag="sc", name="p01")
                    for i, (ot, w) in enumerate(chunks):
                        nc.tensor.matmul(p01[:, 0, :], ot[:, ss], w[:, 0:512],
                                         start=(i == 0), stop=(i == 2))
                    for i, (ot, w) in enumerate(chunks):
                        nc.tensor.matmul(p01[:, 1, :], ot[:, ss],
                                         w[:, 512:1024],
                                         start=(i == 0), stop=(i == 2))
                    p2t = scp.tile([128, 2, SC], F32, tag="sc", name="p2t")
                    for i, (ot, w) in enumerate(chunks):
                        nc.tensor.matmul(p2t[:, 0, 0:256], ot[:, ss],
                                         w[:, 1024:1280],
                                         start=(i == 0), stop=(i == 2))
                    nc.scalar.activation(ost[:, 0:512], p01[:, 0, :], Copy)
                    nc.scalar.activation(ost[:, 512:1024], p01[:, 1, :], Copy)
                    nc.vector.tensor_copy(ost[:, 1024:1280], p2t[:, 0, 0:256])
                    nc.sync.dma_start(out[row:row + 128, :], ost[:])

            for qc in range(NSC):
                qs = slice(qc * SC, (qc + 1) * SC)
                oa = [oap.tile([128, SC], F32, tag=f"oa{h}", name=f"oa{h}")
                      for h in range(2)]
                ob = [oap.tile([128, SC], F32, tag=f"ob{h}", name=f"ob{h}")
                      for h in range(2)]
                exs = {}

                def emit_pv(j):
                    for h in range(2):
                        nc.tensor.matmul(oa[h][:], V[h][:, j, 0:128],
                                         exs[j][:, h, :],
                                         start=(j == 0), stop=(j == NJ - 1))
                        nc.tensor.matmul(ob[h][:], V[h][:, j, 128:256],
                                         exs[j][:, h, :],
                                         start=(j == 0), stop=(j == NJ - 1))

                for j in range(NJ):
                    js = slice(j * 128, (j + 1) * 128)
                    sc_ps = scp.tile([128, 2, SC], F32, tag="sc", name="sc")
                    for h in range(2):
                        nc.tensor.matmul(sc_ps[:, h, :], KTA[h][:, js],
                                         QTA[h][:, qs],
                                         start=True, stop=False)
                        nc.tensor.matmul(sc_ps[:, h, :], KTB[h][:, js],
                                         QTB[h][:, qs],
                                         start=False, stop=True)
                    if j == 4 and pending_ph3:
                        emit_ph3(*pending_ph3.pop())
                    if j >= 3:
                        emit_pv(j - 3)
                        del exs[j - 3]
                    ex = ep.tile([128, 2, SC], BF16, tag="ex", name="ex")
                    nc.scalar.activation(ex[:], sc_ps[:], Exp,
                                         scale=1.0 / QSCALE)
                    exs[j] = ex
                for j in range(NJ - 3, NJ):
                    emit_pv(j)

                # normalization: broadcast 1/denom and scale into bf16
                # oT chunk tiles (rows match the WOA/WOB chunking)
                for h in range(2):
                    nc.vector.tensor_copy(den[h][0:1, :], ob[h][32:33, :])
                rbdt = scp.tile([128, 2, SC], F32, tag="sc", name="rbdt")
                for h in range(2):
                    nc.tensor.matmul(rbdt[:, h, :], ones2[:], den[h][:],
                                     start=True, stop=True)
                rbs = nsb.tile([128, 2, SC], F32, tag="rbs", name="rbs")
                nc.vector.reciprocal_approx_fast(rbs[:], rbdt[:])
                OTA = [nsb.tile([128, SC], BF16, tag=f"ota{h}",
                                name=f"OTA{h}") for h in range(2)]
                OTB = nsb.tile([128, SC], BF16, tag="otb", name="OTB")
                nc.vector.memset(OTB[64:128, :], 0.0)
                nc.vector.tensor_mul(OTA[0][:], oa[0][:], rbs[:, 0, :])
                nc.vector.tensor_mul(OTA[1][:], oa[1][:], rbs[:, 1, :])
                nc.vector.tensor_mul(OTB[0:32, :], ob[0][0:32, :],
                                     rbs[0:32, 0, :])
                nc.vector.tensor_mul(OTB[32:64, :], ob[1][0:32, :],
                                     rbs[0:32, 1, :])

                # output projection is deferred into the next q-chunk's
                # j-loop so the PE stays busy while DVE finishes the muls
                pending_ph3.append((OTA, OTB, qc))
            while pending_ph3:
                emit_ph3(*pending_ph3.pop())

    nc.compile()
    return nc


def _interleave(w):
    """[D, n] -> [128, 10, n] with out[p, c, :] = w[c*128 + p, :]."""
    n = w.shape[1]
    return np.ascontiguousarray(
        w.reshape(NCK, CK, n).transpose(1, 0, 2))


def kernel(hidden_states, w_q, w_k, w_v, lora_k_a, lora_k_b,
           lora_v_a, lora_v_b, w_out, b_out):
    f64 = np.float64
    bf16 = ml_dtypes.bfloat16
    wk_eff = (w_k.astype(f64)
              + w_k.astype(f64) @ lora_k_a.astype(f64) @ lora_k_b.astype(f64))
    wv_eff = (w_v.astype(f64)
              + w_v.astype(f64) @ lora_v_a.astype(f64) @ lora_v_b.astype(f64))
    wq_s = w_q.astype(f64) * (QSCALE / np.sqrt(DH))

    xT = [np.ascontiguousarray(np.asarray(hidden_states)[b].T.astype(bf16)
                               .reshape(NCK, CK, S).transpose(1, 0, 2))
          for b in range(B)]

    in_maps = []
    for c in range(N_CORES):
        b, p = c // 4, c % 4
        cols = slice(p * HP, (p + 1) * HP)
        wq_c = wq_s[:, cols]
        wk_c = wk_eff[:, cols]
        in_maps.append({
            "xT": xT[b],
            "wqm": _interleave(np.concatenate(
                [wq_c[:, 0:128], wq_c[:, 160:288]], axis=1).astype(bf16)),
            "wkm": _interleave(np.concatenate(
                [wk_c[:, 0:128], wk_c[:, 160:288]], axis=1).astype(bf16)),
            "wt": _interleave(np.concatenate(
                [wq_c[:, 128:160], wq_c[:, 288:320],
                 wk_c[:, 128:160], wk_c[:, 288:320]], axis=1).astype(bf16)),
            "wv": _interleave(wv_eff[:, cols].astype(bf16)),
            "wo": np.ascontiguousarray(
                w_out.astype(f64)[cols, :].astype(bf16)),
        })

    global _last_in_maps
    _last_in_maps = in_maps
    if "nc" not in _CACHE:
        _CACHE["nc"] = build()
    res = run_bass_kernel_spmd(_CACHE["nc"], in_maps, list(range(N_CORES)))

    out = np.zeros((B, S, D), np.float32)
    for c in range(N_CORES):
        out[c // 4] += res.results[c]["out"].astype(np.float32)
    out += np.asarray(b_out, np.float32)
    return out
